# revision 1
# baseline (speedup 1.0000x reference)
"""v2: prologue transposes interleaved into the steady state.

Only K(b0)+Q(h0) (2T tiles) are transposed up front; each later head's Q (and
K(b1)) is transposed during the previous head's compute, one tile per O-group
slot, targeting the just-freed exp psum region. All counters are recorded in
a schedule pass and used as exact semaphore wait values.
"""
import numpy as np
import concourse.bass as bass
from concourse import mybir
from contextlib import ExitStack

F32 = mybir.dt.float32
F16 = mybir.dt.float16
EXP = mybir.ActivationFunctionType.Exp
SCALE = float(1.0 / np.sqrt(128.0))

N_CORES = 8


def build_attention_nc(SEQ=2048, B=2, G=4):
    D = 128
    T = SEQ // 128
    QCT = 1
    KG = min(8, T)
    NKP = T // KG
    QC = QCT * 128
    NQC = T // QCT
    H = B * G
    W = KG * QC
    SC, OC = KG, KG * QCT
    NG = H * NQC * NKP
    NQ = H * NQC
    GPH = NQC * NKP            # groups per head
    assert 3 * W + 2 * 512 <= 4096 and QCT == 1

    nc = bass.Bass()
    q_ext = nc.declare_dram_parameter("query", [SEQ, B, G, D], F32, isOutput=False)
    k_ext = nc.declare_dram_parameter("key", [SEQ, B, D], F32, isOutput=False)
    v_ext = nc.declare_dram_parameter("value", [SEQ, B, D], F32, isOutput=False)
    o_ext = nc.declare_dram_parameter("out", [SEQ, B, G, D], F32, isOutput=True)

    # loads in first-use order
    loads = [("K", 0, None)] + [("Q", 0, g) for g in range(G)]
    if B > 1:
        loads += [("K", 1, None)] + [("Q", 1, g) for g in range(G)]
    NL = len(loads)
    N_TR = T * NL

    def q_load_index(h):
        b, g = divmod(h, G)
        return b * (G + 1) + 1 + g

    # ---------------- schedule pass ----------------
    # transposes due during head h (for head h+1)
    due = {h: [] for h in range(H)}
    for nh in range(1, H):
        if nh % G == 0:
            i = nh // G * (G + 1)              # K(b) load: split 2 heads early
            js = list(range(i * T, (i + 1) * T))
            due[max(0, nh - 3)].extend(js[:T // 2])
            due[max(0, nh - 2)].extend(js[T // 2:])
        i = q_load_index(nh)
        due[nh - 1].extend(range(i * T, (i + 1) * T))

    sched = []                      # ("tr", j, col_block) | ("S", e) | ("O", e)
    qc_tr_count = {}
    if GPH < 8 or NKP < 2:
        # small configs: full up-front prologue (v1 style)
        for h in range(H):
            due[h] = []
        init_js = list(range(N_TR))
    else:
        init_js = list(range(2 * T))
    # per-qc tr slot counter (<= 4 per qc: idle opsum buf has 4 x 128 cols)
    steady = []
    for e in range(NG):
        steady.append(("S", e))
        if e >= 2:
            ep = e - 2
            steady.append(("O", ep))
            h_prev = (ep // NKP) // NQC
            Qi_p = ep // NKP
            kp_p = ep % NKP
            slot_in_head = ep - h_prev * GPH
            slots_left = 0
            if kp_p >= 1 and (slot_in_head < GPH - NKP):
                rem = 0
                for ee in range(ep, h_prev * GPH + GPH - NKP):
                    if ee % NKP >= 1:
                        rem += 1
                slots_left = rem
            dl = due[h_prev]
            if dl and slots_left >= 1 and kp_p >= 1:
                # <=4 trs per qc (bank A of the idle opsum buf = 4 blocks);
                # copies are batched after the qc's last tr, so PE never
                # rewrites this bank while DVE reads it (P10 safety)
                used = qc_tr_count.get(Qi_p, 0)
                n_emit = max(min(len(dl), 2, 4 - used), 0)
            else:
                n_emit = 0
            for k in range(n_emit):
                blk = qc_tr_count.get(Qi_p, 0)
                qc_tr_count[Qi_p] = blk + 1
                steady.append(("tr", dl.pop(0), blk))
    steady.append(("O", NG - 2))
    steady.append(("O", NG - 1))
    # any unplaceable trs -> fall back to a full up-front prologue (late-load
    # trs in the initial phase would deadlock against steady-gated DMAs)
    if any(due.values()):
        init_js = list(range(N_TR))
        steady = [op for op in steady if op[0] != "tr"]
        qc_tr_count = {}
    init_js.sort()
    for pos, j in enumerate(init_js):
        sched.append(("tr", j, pos % 8))
    sched.extend(steady)
    N_INIT = len(init_js)
    init_pos = {j: pos for pos, j in enumerate(init_js)}
    init_js_by_pos = init_js

    # walk: assign pe positions; dve emission order & counts
    pe_after_tr = {}
    pe_after_S = {}
    pe_after_O = {}
    tr_at_slot = {}        # group e -> list of (j, col_block) emitted right after O(e)
    pe = 0
    cur_slot = None
    for op in sched:
        if op[0] == "tr":
            pe += 1
            pe_after_tr[op[1]] = pe
            if cur_slot is not None:
                tr_at_slot.setdefault(cur_slot, []).append((op[1], op[2]))
        elif op[0] == "S":
            pe += SC
            pe_after_S[op[1]] = pe
            cur_slot = None
        else:
            pe += OC
            pe_after_O[op[1]] = pe
            cur_slot = op[1]

    # DVE order: copies follow their tr in sched order; norm blocks after the
    # O of each Qi's last group. Build dve op list and record counts.
    dve_ops = []   # ("cp", j, col) | ("cp4", j0, col0) | ("norm", Qi)
    pend_cp = []
    pend4 = []
    in_steady = False

    def flush4():
        if len(pend4) == 4:
            js = [p[0] for p in pend4]
            cols = [p[1] for p in pend4]
            if (js == list(range(js[0], js[0] + 4))
                    and js[0] // T == js[3] // T and js[0] % T + 4 <= T
                    and cols == list(range(cols[0], cols[0] + 4))
                    and cols[0] % 4 == 0):
                dve_ops.append(("cp4", js[0], cols[0]))
            else:
                dve_ops.extend(("cp", j, c) for j, c in pend4)
            pend4.clear()

    for op in sched:
        if op[0] == "tr":
            if in_steady:
                pend_cp.append(("cp", op[1], op[2]))
            else:
                pend4.append((op[1], op[2]))
                flush4()
        elif op[0] == "S":
            if not in_steady:
                dve_ops.extend(("cp", j, c) for j, c in pend4)
                pend4.clear()
            in_steady = True
        elif op[0] == "O":
            e = op[1]
            if e % NKP == NKP - 1:
                dve_ops.extend(pend_cp)
                pend_cp = []
                dve_ops.append(("norm", e // NKP))
    dve_ops.extend(pend_cp)
    copy_done = {}
    recips_done = {}
    mults_done = {}
    dve = 0
    for op in dve_ops:
        if op[0] == "cp":
            dve += 1
            copy_done[op[1]] = dve
        elif op[0] == "cp4":
            dve += 1
            for j in range(op[1], op[1] + 4):
                copy_done[j] = dve
        else:
            dve += QCT
            recips_done[op[1]] = dve
            dve += QCT
            mults_done[op[1]] = dve

    # last tr PE position per qc (steady cp batches wait for it: P10 —
    # a copy must not read bank A while a later tr of the same qc writes it)
    qc_tr_last_pe = {}
    for e, trs in tr_at_slot.items():
        Qi = e // NKP
        for j, _ in trs:
            qc_tr_last_pe[Qi] = max(qc_tr_last_pe.get(Qi, 0), pe_after_tr[j])

    # last PE position among a load's transposes (for qnat reuse gating)
    load_last_pe = {i: max(pe_after_tr[j] for j in range(i * T, (i + 1) * T))
                    for i in range(NL)}

    # per-head "all Q/K transposes copied" values for first S of head
    head_ready = {}
    for h in range(H):
        js = list(range(q_load_index(h) * T, (q_load_index(h) + 1) * T))
        b = h // G
        kload = b * (G + 1)
        js += list(range(kload * T, (kload + 1) * T))
        head_ready[h] = max(copy_done[j] for j in js)

    def eidx(e):
        kp = e % NKP
        Qi = e // NKP
        qc = Qi % NQC
        h = Qi // NQC
        return h, qc, kp, Qi

    # ---------------- tensors ----------------
    ident = nc.alloc_sbuf_tensor("ident", [128, 128], F32)
    bias0 = nc.alloc_sbuf_tensor("bias0", [128, 1], F32)
    qnat = [nc.alloc_sbuf_tensor(f"qnat{i}", [128, T * 128], F32) for i in range(3)]
    KT = [nc.alloc_sbuf_tensor(f"KT{b}", [128, T * 128], F16) for b in range(B)]
    QT = [nc.alloc_sbuf_tensor(f"QT{h}", [128, T * 128], F16) for h in range(H)]
    VT = [nc.alloc_sbuf_tensor(f"VT{b}", [128, T * 132], F16) for b in range(B)]
    PT = [nc.alloc_sbuf_tensor(f"PT{s}", [128, W], F16) for s in range(3)]
    rsb = [nc.alloc_sbuf_tensor(f"rsb{s}", [128, QCT], F32) for s in range(2)]
    OS = [nc.alloc_sbuf_tensor(f"OS{s}", [128, T * 128], F32) for s in range(2)]
    psum = nc.alloc_psum_tensor("psum", [128, 4096], F32)

    def spsum(s):
        return psum[:, s * W:(s + 1) * W]

    def opsum(buf, qs):
        assert qs == 0
        off = 3072 + buf * 512
        return psum[:, off:off + 129]

    def tr_psum(e_slot, col):
        if e_slot is None:
            # initial phase: 8 bank-aligned slots (P10: a trailing DVE copy
            # must never read the bank a new PE transpose is writing)
            return psum[:, 512 * (col % 8):512 * (col % 8) + 128]
        Qi = e_slot // NKP
        idle_buf = (Qi + 1) % 2
        off = 3072 + idle_buf * 512 + 128 * (col % 4)
        return psum[:, off:off + 128]

    def tr_dest(j):
        i, jl = divmod(j, T)
        kind, b, g = loads[i]
        t = KT[b] if kind == "K" else QT[b * G + g]
        return t[:, jl * 128:(jl + 1) * 128]

    with ExitStack() as ctx:
        sem_pe = ctx.enter_context(nc.semaphore("sem_pe"))
        sem_act = ctx.enter_context(nc.semaphore("sem_act"))
        sem_dve = ctx.enter_context(nc.semaphore("sem_dve"))
        sem_pool = ctx.enter_context(nc.semaphore("sem_pool"))
        sem_load = [ctx.enter_context(nc.semaphore(f"sem_load{i}"))
                    for i in range(NL)]
        sem_out = [ctx.enter_context(nc.semaphore(f"sem_out{h}"))
                   for h in range(H)]
        sem_v = [ctx.enter_context(nc.semaphore(f"sem_v{b}")) for b in range(B)]
        block = ctx.enter_context(nc.Block())

        @block.sync
        def _(sync):
            for i, (kind, b, g) in enumerate(loads):
                if i >= 3:
                    nc.sync.wait_ge(sem_pe, load_last_pe[i - 3])
                src = k_ext[:, b, :] if kind == "K" else q_ext[:, b, g, :]
                nc.sync.dma_start(
                    out=qnat[i % 3][:].rearrange("p (t d) -> p t d", d=128),
                    in_=src.rearrange("(t p) d -> p t d", p=128),
                ).then_inc(sem_load[i], 16)
            for h in range(H):
                nc.sync.wait_ge(sem_out[h], 32)

        @block.gpsimd
        def _(gp):
            nc.gpsimd.memset(ident[:], 0.0).then_inc(sem_pool)
            nc.gpsimd.wait_ge(sem_pool, 1)
            nc.gpsimd.affine_select(
                out=ident[:], in_=ident[:],
                compare_op=mybir.AluOpType.not_equal, fill=1.0,
                base=0, pattern=[[-1, 128]], channel_multiplier=1,
            ).then_inc(sem_pool)
            nc.gpsimd.memset(bias0[:], 0.0).then_inc(sem_pool)
            for b in range(B):
                vt3 = VT[b][:].rearrange("p (t c) -> p t c", c=132)
                nc.gpsimd.memset(vt3[:, :, 128:129], 1.0).then_inc(sem_pool)
                nc.gpsimd.dma_start(
                    out=vt3[:, :, 0:128],
                    in_=v_ext[:, b, :].rearrange("(t p) d -> p t d", p=128),
                ).then_inc(sem_v[b], 16)
            # output stores on the SWDGE queue (SP would head-of-line block
            # behind late-gated input loads)
            for h in range(H):
                b, g = divmod(h, G)
                half = NQC // 2
                oh = o_ext[:, b, g, :].rearrange("(t p) d -> p t d", p=128)
                osh = OS[h % 2][:].rearrange("p (t d) -> p t d", d=128)
                nc.gpsimd.wait_ge(sem_dve, mults_done[h * NQC + half - 1])
                nc.gpsimd.dma_start(
                    out=oh[:, 0:half, :], in_=osh[:, 0:half, :],
                ).then_inc(sem_out[h], 16)
                nc.gpsimd.wait_ge(sem_dve, mults_done[h * NQC + NQC - 1])
                nc.gpsimd.dma_start(
                    out=oh[:, half:NQC, :], in_=osh[:, half:NQC, :],
                ).then_inc(sem_out[h], 16)

        @block.tensor
        def _(te):
            nc.tensor.wait_ge(sem_pool, 2)
            cur_slot = [None]

            seen_qc_tr = set()
            seen_loads = set()

            def emit_tr(j, col):
                ld = j // T
                if ld not in seen_loads:
                    seen_loads.add(ld)
                    nc.tensor.wait_ge(sem_load[ld], 16)
                if cur_slot[0] is None:
                    pos = init_pos[j]
                    if pos % 8 == 0 and pos >= 8:
                        nc.tensor.wait_ge(
                            sem_dve, copy_done[init_js_by_pos[pos - 1]])
                if cur_slot[0] is not None:
                    Qi = cur_slot[0] // NKP
                    if Qi not in seen_qc_tr and Qi >= 1:
                        seen_qc_tr.add(Qi)
                        nc.tensor.wait_ge(sem_dve, mults_done[Qi - 1])
                nc.tensor.transpose(
                    tr_psum(cur_slot[0], col),
                    qnat[(j // T) % 3][:, (j % T) * 128:(j % T + 1) * 128],
                    ident[:],
                ).then_inc(sem_pe)

            def emit_S(e):
                h, qc, kp, Qi = eidx(e)
                b = h // G
                act_w = e - 2 if e >= 3 else None
                init_done = copy_done[init_js_by_pos[-1]] if init_js_by_pos else 0
                if e % GPH == 0:              # first S of head h
                    nc.tensor.wait_ge(sem_dve, max(head_ready[h],
                                                   init_done if e < 3 else 0))
                elif e < 3:
                    nc.tensor.wait_ge(sem_dve, max(head_ready[0], init_done))
                s = e % 3
                for ki in range(KG):
                    kt = kp * KG + ki
                    inst = nc.tensor.matmul(
                        spsum(s)[:, ki * QC:(ki + 1) * QC],
                        KT[b][:, kt * 128:(kt + 1) * 128],
                        QT[h][:, qc * QC:(qc + 1) * QC],
                        start=True, stop=True, skip_group_check=True,
                    )
                    if ki == 0 and act_w is not None:
                        inst._wait_ge(sem_act, act_w)
                    inst.then_inc(sem_pe)
                cur_slot[0] = None

            def emit_O(e):
                h, qc, kp, Qi = eidx(e)
                b = h // G
                s = e % 3
                o_first = [True]
                if kp == 0:
                    w = 0
                    if Qi >= 2:
                        w = mults_done[Qi - 2]
                    if Qi >= 1:
                        for ee in range((Qi - 1) * NKP, Qi * NKP):
                            for j, _ in tr_at_slot.get(ee, []):
                                w = max(w, copy_done[j])
                    if w:
                        nc.tensor.wait_ge(sem_dve, w)
                    if e == b * G * GPH:
                        nc.tensor.wait_ge(sem_v[b], 16)
                        nc.tensor.wait_ge(sem_pool, 4 + b)
                buf = Qi % 2
                vt3 = VT[b][:].rearrange("p (t c) -> p t c", c=132)
                for ki in range(KG):
                    kt = kp * KG + ki
                    for qs in range(QCT):
                        inst = nc.tensor.matmul(
                            opsum(buf, qs)[:, 0:129],
                            PT[s][:, ki * QC + qs * 128:ki * QC + qs * 128 + 128],
                            vt3[:, kt, 0:129],
                            start=(kt == 0), stop=(kt == T - 1),
                            skip_group_check=True,
                        )
                        if o_first[0]:
                            o_first[0] = False
                            inst._wait_ge(sem_act, e + 1)
                        inst.then_inc(sem_pe)
                cur_slot[0] = e

            for op in sched:
                if op[0] == "tr":
                    emit_tr(op[1], op[2])
                elif op[0] == "S":
                    emit_S(op[1])
                else:
                    emit_O(op[1])

        @block.scalar
        def _(sc):
            nc.scalar.wait_ge(sem_pool, 3)
            for e in range(NG):
                s = e % 3
                # wait carried in the activation's own sync_info (capacity 1)
                # instead of a standalone EventSemaphore: 256 fewer ACT issues
                nc.scalar.activation(
                    out=PT[s][:, 0:W], in_=spsum(s),
                    func=EXP, bias=bias0[:, 0:1], scale=SCALE,
                )._wait_ge(sem_pe, pe_after_S[e]).then_inc(sem_act)

        @block.vector
        def _(ve):
            cur_slot = [None]
            first_steady = [False]

            def emit_cp(j, col):
                if cur_slot[0] is not None:
                    Qi = cur_slot[0] // NKP
                    nc.vector.wait_ge(sem_pe, qc_tr_last_pe[Qi])
                else:
                    nc.vector.wait_ge(sem_pe, pe_after_tr[j])
                nc.vector.tensor_copy(
                    tr_dest(j), tr_psum(cur_slot[0], col)).then_inc(sem_dve)

            def emit_norm(Qi):
                h, qc = divmod(Qi, NQC)
                buf = Qi % 2
                e_last = Qi * NKP + NKP - 1
                nc.vector.wait_ge(sem_pe, pe_after_O[e_last])
                if Qi >= 2:
                    nc.vector.wait_ge(sem_dve, mults_done[Qi - 2])  # rsb WAR edge
                for qs in range(QCT):
                    nc.vector.reciprocal(
                        rsb[buf][:, qs:qs + 1], opsum(buf, qs)[:, 128:129]
                    ).then_inc(sem_dve)
                nc.vector.wait_ge(sem_dve, recips_done[Qi])  # rsb RAW drain
                if qc == 0 and h >= 2:
                    nc.vector.wait_ge(sem_out[h - 2], 32)
                for qs in range(QCT):
                    nc.vector.tensor_scalar(
                        OS[h % 2][:, (qc * QCT + qs) * 128:(qc * QCT + qs + 1) * 128],
                        opsum(buf, qs)[:, 0:128],
                        rsb[buf][:, qs:qs + 1],
                        None,
                        op0=mybir.AluOpType.mult,
                    ).then_inc(sem_dve)

            # replay in dve order, tracking the psum slot of each tr
            it_slot = {}
            cs = None
            for op in sched:
                if op[0] == "tr":
                    it_slot[op[1]] = cs
                elif op[0] == "O":
                    cs = op[1]
                elif op[0] == "S":
                    cs = None
            for op in dve_ops:
                if op[0] == "cp":
                    cur_slot[0] = it_slot[op[1]]
                    emit_cp(op[1], op[2])
                elif op[0] == "cp4":
                    j0, c0 = op[1], op[2]
                    nc.vector.wait_ge(sem_pe, pe_after_tr[j0 + 3])
                    li, jl0 = divmod(j0, T)
                    kind, lb, lg = loads[li]
                    t = KT[lb] if kind == "K" else QT[lb * G + lg]
                    src4 = psum[:, 512 * c0:512 * c0 + 2048].rearrange(
                        "p (s c) -> p s c", c=512)[:, :, 0:128]
                    dst4 = t[:, jl0 * 128:(jl0 + 4) * 128].rearrange(
                        "p (s c) -> p s c", c=128)
                    nc.vector.tensor_copy(dst4, src4).then_inc(sem_dve)
                else:
                    emit_norm(op[1])

    return nc


_NC = None


def _get_nc():
    global _NC
    if _NC is None:
        _NC = build_attention_nc(2048, 2, 4)
    return _NC


def kernel(query, key, value):
    from concourse.bass_utils import run_bass_kernel_spmd

    query = np.ascontiguousarray(query, dtype=np.float32)
    key = np.ascontiguousarray(key, dtype=np.float32)
    value = np.ascontiguousarray(value, dtype=np.float32)
    G = query.shape[2] // key.shape[2]
    nc = _get_nc()
    in_maps = []
    for c in range(N_CORES):
        in_maps.append({
            "query": np.ascontiguousarray(query[:, :, c * G:(c + 1) * G, :]),
            "key": np.ascontiguousarray(key[:, :, c, :]),
            "value": np.ascontiguousarray(value[:, :, c, :]),
        })
    res = run_bass_kernel_spmd(nc, in_maps, list(range(N_CORES)))
    out = np.empty_like(query)
    for c in range(N_CORES):
        out[:, :, c * G:(c + 1) * G, :] = res.results[c]["out"]
    return out

